# revision 6
# baseline (speedup 1.0000x reference)
"""LinearAttention Trainium2 kernel — batch-parallel over 8 NeuronCores.

Math (per batch b, reference semantics):
  qkv = w_qkv @ x            # [384, n], n = 64*64 = 4096
  q = softmax_d(qkv[0:128]) * 32**-0.5     (softmax over feature dim within each head)
  k = softmax_n(qkv[128:256])              (softmax over spatial dim)
  v = qkv[256:384]
  ctx = k @ v.T per head; out = ctx.T @ q  # linear attention
  out = w_out @ out + b_out
  out = out / ||out||_c * g * 16           # RMS over channels

Tricks used (all divisions commute out of the contractions):
  - k-softmax: ctx_raw = exp(k) @ [v|1].T accumulated on PE; the |1 column gives
    T[d] = sum_n exp(k); ctx = ctx_raw * (1/T) as a per-partition scalar.
  - q-softmax: S[h,n] = sum_d exp(q) broadcast to all 128 rows via a
    block-diagonal ones matmul; attn = (ctx_masked @ exp(q)) / S elementwise.
  - rsqrt for RMS = exp(-0.5*ln(x)) so ACT uses one table set (exp+ln+copy).
  - out-proj emitted transposed ([n, c]) so RMS reduce is a free-dim accum;
    host transposes the output back.
"""

import numpy as np

import concourse.bass as bass
import concourse.mybir as mybir
import concourse.tile as tile
from concourse.bass_utils import run_bass_kernel_spmd

HEADS, DH = 4, 32
B, C, H, W = 16, 256, 64, 64
N = H * W                      # 4096
NCORES = 8
BPC = B // NCORES              # batches per core
HID = HEADS * DH               # 128
SCALE = DH ** -0.5
NT = N // 128                  # 32 n-tiles
NCH = N // 512                 # 8 chunks
F32 = mybir.dt.float32
AF = mybir.ActivationFunctionType
ALU = mybir.AluOpType


def _split_waits(nc, max_waits=1):
    """This walrus build rejects >1 sync wait per TPB_CTRL instruction; hoist
    excess waits onto preceding NoOps (engines execute in order, so semantics
    are unchanged)."""
    for f in nc.m.functions:
        for bb in f.blocks:
            new = []
            for ins in bb.instructions:
                si = getattr(ins, "sync_info", None)
                if si is not None and si.on_wait and len(si.on_wait) > max_waits:
                    extra = list(si.on_wait[:-max_waits])
                    si.on_wait = list(si.on_wait[-max_waits:])
                    for k, w in enumerate(extra):
                        nop = mybir.InstNoOp(
                            name=f"{ins.name}-wsplit{k}", ins=[], outs=[],
                            sync_info=mybir.SyncInfo(on_wait=[w], on_update=[]))
                        nop.engine = ins.engine
                        new.append(nop)
                new.append(ins)
            bb.instructions = new


def _build_nc():
    nc = bass.Bass("TRN2", target_bir_lowering=False, debug=False)
    x_d = nc.declare_dram_parameter("x", [BPC, C, N], F32, isOutput=False)
    wqkvT_d = nc.declare_dram_parameter("wqkvT", [C, 3 * HID], F32, isOutput=False)
    woT_d = nc.declare_dram_parameter("woT", [HID, C], F32, isOutput=False)
    bC_d = nc.declare_dram_parameter("bC", [128, 2, 256], F32, isOutput=False)
    gC_d = nc.declare_dram_parameter("gC", [128, 2, 256], F32, isOutput=False)
    maskS_d = nc.declare_dram_parameter("maskS", [128, 128], F32, isOutput=False)
    maskE_d = nc.declare_dram_parameter("maskE", [128, 128], F32, isOutput=False)
    y_d = nc.declare_dram_parameter("y", [BPC, N, C], F32, isOutput=True)

    with tile.TileContext(nc) as tc:
        with (
            tc.tile_pool(name="const", bufs=1) as constp,
            tc.tile_pool(name="xp", bufs=2) as xp,
            tc.tile_pool(name="kvp_sb", bufs=1) as kvsb,
            tc.tile_pool(name="attn", bufs=2) as attnp,
            tc.tile_pool(name="small", bufs=2) as smallp,
            tc.tile_pool(name="eqp", bufs=3) as eqp,
            tc.tile_pool(name="sps", bufs=3) as spsb,
            tc.tile_pool(name="ocp", bufs=2) as ocp,
            tc.tile_pool(name="sqp", bufs=2) as sqp,
            tc.tile_pool(name="finp", bufs=3) as finp,
            tc.tile_pool(name="ps_kv", bufs=2, space="PSUM") as ps_kv,
            tc.tile_pool(name="ps_ctx", bufs=1, space="PSUM") as ps_ctx,
            tc.tile_pool(name="ps_q", bufs=1, space="PSUM") as ps_q,
            tc.tile_pool(name="ps_s", bufs=1, space="PSUM") as ps_s,
            tc.tile_pool(name="ps_e", bufs=1, space="PSUM") as ps_e,
            tc.tile_pool(name="ps_op", bufs=2, space="PSUM") as ps_op,
        ):
            # ---- constants ----
            wqkvT = constp.tile([128, 2, 3 * HID], F32)
            nc.sync.dma_start(wqkvT[:], wqkvT_d.rearrange("(b p) o -> p b o", p=128))
            woT = constp.tile([128, C], F32)
            nc.sync.dma_start(woT[:], woT_d[:])
            bC = constp.tile([128, 2, 256], F32)
            nc.sync.dma_start(bC[:], bC_d[:])
            gC = constp.tile([128, 2, 256], F32)
            nc.sync.dma_start(gC[:], gC_d[:])
            maskS = constp.tile([128, 128], F32)
            nc.sync.dma_start(maskS[:], maskS_d[:])
            maskE = constp.tile([128, 128], F32)
            nc.sync.dma_start(maskE[:], maskE_d[:])

            for b in range(BPC):
                # ---- load x: [128, cblk, n] ----
                x_t = xp.tile([128, 2, N], F32)
                nc.sync.dma_start(x_t[:], x_d[b].rearrange("(b p) n -> p b n", p=128))

                # ---- kv projection, transposed layout [n, k|v|1] ----
                kv_t = kvsb.tile([128, NT, 257], F32)
                nc.gpsimd.memset(kv_t[:, :, 256:257], 1.0)
                for r in range(NT // 2):
                    kvps = ps_kv.tile([128, 2, 256], F32)
                    for i in range(2):
                        t = 2 * r + i
                        nc.tensor.matmul(
                            kvps[:, i, :], x_t[:, 0, t * 128:(t + 1) * 128],
                            wqkvT[:, 0, HID:3 * HID], start=True, stop=False)
                        nc.tensor.matmul(
                            kvps[:, i, :], x_t[:, 1, t * 128:(t + 1) * 128],
                            wqkvT[:, 1, HID:3 * HID], start=False, stop=True)
                    nc.scalar.activation(
                        kv_t[:, 2 * r:2 * r + 2, 0:128], kvps[:, :, 0:128], AF.Exp)
                    nc.scalar.copy(
                        kv_t[:, 2 * r:2 * r + 2, 128:256], kvps[:, :, 128:256])

                # ---- context (+T in col 128): accumulate over n-tiles ----
                ctxps = ps_ctx.tile([128, 129], F32)
                for t in range(NT):
                    nc.tensor.matmul(
                        ctxps[:], kv_t[:, t, 0:128], kv_t[:, t, 128:257],
                        start=(t == 0), stop=(t == NT - 1))
                recipT = smallp.tile([128, 1], F32)
                nc.vector.reciprocal(recipT[:], ctxps[:, 128:129])
                cm = smallp.tile([128, 128], F32)
                nc.vector.tensor_scalar(cm[:], ctxps[:, 0:128], recipT[:], None, ALU.mult)
                nc.vector.tensor_tensor(cm[:], cm[:], maskE[:], ALU.mult)

                # ---- q proj + softmax normalizer + einsum2, per 512-chunk ----
                attn = attnp.tile([128, N], F32)
                for ch in range(NCH):
                    sl = slice(ch * 512, (ch + 1) * 512)
                    qps = ps_q.tile([128, 512], F32)
                    nc.tensor.matmul(qps[:], wqkvT[:, 0, 0:HID], x_t[:, 0, sl],
                                     start=True, stop=False)
                    nc.tensor.matmul(qps[:], wqkvT[:, 1, 0:HID], x_t[:, 1, sl],
                                     start=False, stop=True)
                    eq = eqp.tile([128, 512], F32)
                    nc.scalar.activation(eq[:], qps[:], AF.Exp)
                    sps = ps_s.tile([128, 512], F32)
                    nc.tensor.matmul(sps[:], maskS[:], eq[:], start=True, stop=True)
                    eps = ps_e.tile([128, 512], F32)
                    nc.tensor.matmul(eps[:], cm[:], eq[:], start=True, stop=True)
                    s_sb = spsb.tile([128, 512], F32)
                    nc.vector.reciprocal(s_sb[:], sps[:])
                    nc.vector.tensor_tensor(attn[:, sl], eps[:], s_sb[:], ALU.mult)

                # ---- tail: out-proj transposed + bias + RMS + store ----
                for gidx in range(4):
                    oc = ocp.tile([128, 8, 256], F32)
                    nsq = smallp.tile([128, 8], F32)
                    for r4 in range(4):
                        r = 4 * gidx + r4
                        ops = ps_op.tile([128, 2, 256], F32)
                        for i in range(2):
                            t = 2 * r + i
                            nc.tensor.matmul(
                                ops[:, i, :], attn[:, t * 128:(t + 1) * 128],
                                woT[:], start=True, stop=True)
                        nc.vector.tensor_tensor(
                            oc[:, 2 * r4:2 * r4 + 2, :], ops[:], bC[:], ALU.add)
                        for i in range(2):
                            sqs = sqp.tile([128, 256], F32)
                            nc.vector.scalar_tensor_tensor(
                                sqs[:], oc[:, 2 * r4 + i, :], 1.0,
                                oc[:, 2 * r4 + i, :], ALU.mult, ALU.mult,
                                accum_out=nsq[:, 2 * r4 + i:2 * r4 + i + 1])
                    lnn = smallp.tile([128, 8], F32)
                    nc.scalar.activation(lnn[:], nsq[:], AF.Ln)
                    rr = smallp.tile([128, 8], F32)
                    nc.scalar.activation(rr[:], lnn[:], AF.Exp, scale=-0.5)
                    for r4 in range(4):
                        r = 4 * gidx + r4
                        fin = finp.tile([128, 2, 256], F32)
                        for i in range(2):
                            nc.vector.scalar_tensor_tensor(
                                fin[:, i, :], oc[:, 2 * r4 + i, :],
                                rr[:, 2 * r4 + i:2 * r4 + i + 1],
                                gC[:, i, :], ALU.mult, ALU.mult)
                        nc.sync.dma_start(
                            y_d[b].rearrange("(t p) c -> p t c", p=128)[:, 2 * r:2 * r + 2, :],
                            fin[:])
    _split_waits(nc)
    return nc


_NC_CACHE = None


def _get_nc():
    global _NC_CACHE
    if _NC_CACHE is None:
        _NC_CACHE = _build_nc()
    return _NC_CACHE


def kernel(x, w_qkv, w_out, b_out, g):
    x = np.ascontiguousarray(np.asarray(x, dtype=np.float32)).reshape(B, C, N)
    w_qkv = np.asarray(w_qkv, dtype=np.float32)
    w_out = np.asarray(w_out, dtype=np.float32)
    b_out = np.asarray(b_out, dtype=np.float32).reshape(C)
    g = np.asarray(g, dtype=np.float32).reshape(C)

    wqkvT = np.ascontiguousarray(w_qkv.T)                      # [256, 384]
    woT = np.ascontiguousarray(w_out.T)                        # [128, 256]
    bC = np.ascontiguousarray(
        np.broadcast_to(b_out.reshape(1, 1, 256), (128, 2, 256)))
    gC = np.ascontiguousarray(
        np.broadcast_to((g * (C ** 0.5)).reshape(1, 1, 256), (128, 2, 256)))
    blk = np.zeros((128, 128), dtype=np.float32)
    for h in range(HEADS):
        blk[h * DH:(h + 1) * DH, h * DH:(h + 1) * DH] = 1.0
    maskS = blk
    maskE = blk * SCALE

    nc = _get_nc()
    in_maps = []
    for c in range(NCORES):
        in_maps.append({
            "x": np.ascontiguousarray(x[c * BPC:(c + 1) * BPC]),
            "wqkvT": wqkvT, "woT": woT, "bC": bC, "gC": gC,
            "maskS": maskS, "maskE": maskE,
        })
    res = run_bass_kernel_spmd(nc, in_maps, list(range(NCORES)))
    y = np.stack([res.results[i]["y"] for i in range(NCORES)])   # [8, 2, N, C]
    out = y.reshape(B, N, C).transpose(0, 2, 1).reshape(B, C, H, W)
    return np.ascontiguousarray(out)
